# revision 17
# baseline (speedup 1.0000x reference)
"""Trainium2 Bass kernel for the gnn_message_passing agent model.

Strategy (8 NeuronCores, SPMD):
  - Shard down_states/up_states row-wise across 8 cores (N=500000 padded to
    500736 = 8*62592 rows, zero rows are no-ops for all reductions).
  - Per core, stream the 62592x128 D and U chunks through SBUF in 4096-row
    blocks, cast to bf16, and accumulate Gram blocks on the TensorEngine:
        G1 = D^T @ [D | U | 1]   (PSUM, fp32 accumulate)  -> [G_dd | G_du | colsum_d]
        G2 = U^T @ [U | 1]                                -> [G_uu | colsum_u]
    Then  w = S v,  sum(w) = colsum . v,  w @ S = G v  follow from small
    tail matmuls with v = attention @ last_h.
  - The per-parent GRU (AxisCell) chains run replicated on every core in a
    transposed [feature, parent] layout, interleaved into the PE instruction
    stream so they overlap with the memory-bound streaming part.
  - The output head is linear up to the softmax:
        logits = (out2 @ out1_a @ wsum)/total + (out2 @ (out1_r@region + b1) + b2)
    Each core returns za = out2@out1_a@wsum_partial (additive across cores),
    yb (replicated), and total_partial. The host sums the partials
    (the cross-core all-reduce) and applies the final softmax on 128 values.
"""

import numpy as np
import ml_dtypes

BF16 = ml_dtypes.bfloat16

N, H, P, L, NA = 500000, 128, 64, 50, 128
NCORES = 8
NPAD = 62592            # per-core rows = 489 tiles of 128
NTOT = NPAD * NCORES    # 500736
SUB = 128               # rows per matmul sub-tile
BLOCK_B = 64            # sub-tiles per DMA block (8192 rows, 4 MiB fp32 read)
NT = NPAD // SUB        # 489 sub-tiles per core
GRU_EVERY = 9           # emit one GRU step per this many Gram sub-tiles


def _build_program():
    from contextlib import ExitStack

    import concourse.bass as bass
    import concourse.mybir as mybir
    import concourse.tile as tile
    from concourse import bacc
    from concourse.masks import make_identity

    dt = mybir.dt
    f32 = dt.float32
    bf16 = dt.bfloat16
    Act = mybir.ActivationFunctionType
    Alu = mybir.AluOpType

    nc = bacc.Bacc(
        "TRN2", target_bir_lowering=False, debug=False, num_devices=NCORES
    )

    # ---- DRAM I/O ----
    dch = nc.dram_tensor("dch", [NPAD, H], f32, kind="ExternalInput")
    uch = nc.dram_tensor("uch", [NPAD, H], f32, kind="ExternalInput")
    # all small tensors packed host-side into two [128, X] arrays
    WB = L * P + P + 4 * H + 2 * H + 4 * (3 * H) + H + 2   # bf16 pack cols
    WF = 2 * H + 9                                          # f32 pack cols
    wbp_d = nc.dram_tensor("wbp", [H, WB], bf16, kind="ExternalInput")
    wfp_d = nc.dram_tensor("wfp", [H, WF], f32, kind="ExternalInput")
    out_d = nc.dram_tensor("partials", [H, 4], f32, kind="ExternalOutput")

    with tile.TileContext(nc) as tc, ExitStack() as ctx:
        wp = ctx.enter_context(tc.tile_pool(name="wp", bufs=1))
        dp = ctx.enter_context(tc.tile_pool(name="dp", bufs=3))
        gb = ctx.enter_context(tc.tile_pool(name="gb", bufs=1, space="PSUM"))
        pb = ctx.enter_context(tc.tile_pool(name="pb", bufs=2, space="PSUM"))
        ms = ctx.enter_context(tc.tile_pool(name="ms", bufs=2, space="PSUM"))
        sb = ctx.enter_context(tc.tile_pool(name="sb", bufs=3))

        # ---- load small persistent tensors ----
        def wtile(dram, shape, dtype):
            t = wp.tile(shape, dtype, tag=dram.name)
            nc.sync.dma_start(t[:], dram[:].rearrange("a b -> a b") if False else dram[:])
            return t

        wbp = wp.tile([H, WB], bf16, tag="wbp")
        nc.sync.dma_start(wbp[:], wbp_d[:])
        wfp = wp.tile([H, WF], f32, tag="wfp")
        nc.sync.dma_start(wfp[:], wfp_d[:])

        def bseg(n):
            o = bseg.off
            bseg.off += n
            return wbp[:, o:o + n]
        bseg.off = 0
        embT = bseg(L * P)
        h0T = bseg(P)
        awT = bseg(4 * H)
        atwT = bseg(2 * H)
        swT = bseg(3 * H)
        suT = bseg(3 * H)
        stT = bseg(3 * H)
        o1T = bseg(3 * H)
        o2T = bseg(H)
        xpk = bseg(2)

        def fseg(n):
            o = fseg.off
            fseg.off += n
            return wfp[:, o:o + n]
        fseg.off = 0
        attT = fseg(2 * H)
        lasth = fseg(1)
        ab = fseg(2)
        atb = fseg(1)
        swb = fseg(1)
        sub_t = fseg(1)
        stb = fseg(1)
        o1b = fseg(1)
        o2b = fseg(1)

        ident = wp.tile([H, H], f32, tag="ident")
        make_identity(nc, ident[:])
        ones = wp.tile([H, 1], bf16, tag="ones")
        nc.vector.memset(ones[:], 1.0)

        # ---- persistent PSUM accumulators ----
        G1 = gb.tile([H, 2 * H + 1], f32, tag="G1")   # [G_dd | G_du | cs_d]
        G2 = gb.tile([H, H + 1], f32, tag="G2")       # [G_uu | cs_u]

        # ---- GRU chain, emitted as two half-steps interleaved with the
        #      Gram sub-tile stream so the PE never waits on ACT/DVE ----
        st = {"h": h0T, "l": 0, "a": None, "r": None, "z": None, "rp": None}

        def gru_half1():
            l, h_cur = st["l"], st["h"]
            x_l = embT[:, l * P:(l + 1) * P]
            a_ps = pb.tile([H, 2, P], f32, tag="a")
            nc.tensor.matmul(a_ps[:, 0, :], awT[:, 0:H], x_l, start=True, stop=False)
            nc.tensor.matmul(a_ps[:, 0, :], awT[:, 2 * H:3 * H], h_cur[:], start=False, stop=True)
            nc.tensor.matmul(a_ps[:, 1, :], awT[:, H:2 * H], x_l, start=True, stop=False)
            nc.tensor.matmul(a_ps[:, 1, :], awT[:, 3 * H:4 * H], h_cur[:], start=False, stop=True)
            r_bf = sb.tile([H, P], bf16, tag="r")
            nc.scalar.activation(r_bf[:], a_ps[:, 0, :], Act.Sigmoid, bias=ab[:, 0:1])
            rp = sb.tile([H, P], bf16, tag="rp")
            nc.vector.tensor_tensor(rp[:], r_bf[:], h_cur[:], Alu.mult)
            z_bf = sb.tile([H, P], bf16, tag="z")
            nc.scalar.activation(z_bf[:], a_ps[:, 1, :], Act.Sigmoid, bias=ab[:, 1:2])
            st["r"], st["z"], st["rp"] = r_bf, z_bf, rp

        def gru_half2():
            l, h_cur = st["l"], st["h"]
            x_l = embT[:, l * P:(l + 1) * P]
            t_ps = pb.tile([H, P], f32, tag="t")
            nc.tensor.matmul(t_ps[:], atwT[:, 0:H], x_l, start=True, stop=False)
            nc.tensor.matmul(t_ps[:], atwT[:, H:2 * H], st["rp"][:], start=False, stop=True)
            t_bf = sb.tile([H, P], bf16, tag="tb")
            nc.scalar.activation(t_bf[:], t_ps[:], Act.Tanh, bias=atb[:, 0:1])
            # h' = h + z*(t - h)
            tmh = sb.tile([H, P], bf16, tag="tmh")
            nc.vector.tensor_tensor(tmh[:], t_bf[:], h_cur[:], Alu.subtract)
            ztm = sb.tile([H, P], bf16, tag="ztm")
            nc.vector.tensor_tensor(ztm[:], st["z"][:], tmh[:], Alu.mult)
            h_new = sb.tile([H, P], bf16, tag="h")
            nc.vector.tensor_tensor(h_new[:], h_cur[:], ztm[:], Alu.add)
            st["h"] = h_new
            st["l"] += 1

        # schedule: step l half1 at sub-tile l*GRU_EVERY+10, half2 4 later
        # (offset 10 keeps the first blocks' Gram matmuls unblocked while the
        #  packed-weight DMA lands)
        gru_sched = {}
        for l in range(L):
            gru_sched[l * GRU_EVERY + 10] = gru_half1
            gru_sched[l * GRU_EVERY + 14] = gru_half2

        # ---- streaming Gram accumulation ----
        blocks = [(0, 8)]  # small first block so the PE starts early
        s0 = 8 * SUB
        while s0 < NPAD:
            b = min(BLOCK_B, (NPAD - s0) // SUB)
            blocks.append((s0, b))
            s0 += b * SUB

        kidx = 0
        emitted_v = False
        for (s0, B) in blocks:
            # SWDGE DMA with inline fp32->bf16 cast into the packed S tile:
            # S[:, b, :] = [D_b | U_b | 1 1], so each sub-tile needs only two
            # wide matmuls (rhs N=257 and N=129).
            sbf = dp.tile([SUB, B, 2 * H + 2], bf16, tag="s")
            nc.gpsimd.dma_start(
                sbf[:, :, 0:H],
                dch[s0:s0 + B * SUB, :].rearrange("(p b) h -> p b h", p=SUB),
            )
            nc.gpsimd.dma_start(
                sbf[:, :, H:2 * H],
                uch[s0:s0 + B * SUB, :].rearrange("(p b) h -> p b h", p=SUB),
            )
            nc.vector.memset(sbf[:, :, 2 * H:2 * H + 2], 1.0)
            for b in range(B):
                st_, sp_ = kidx == 0, kidx == NT - 1
                nc.tensor.matmul(
                    G1[:, :], sbf[:, b, 0:H], sbf[:, b, 0:2 * H + 1],
                    start=st_, stop=sp_,
                )
                nc.tensor.matmul(
                    G2[:, :], sbf[:, b, H:2 * H], sbf[:, b, H:2 * H + 1],
                    start=st_, stop=sp_,
                )
                act = gru_sched.get(kidx)
                if act is not None:
                    act()
                kidx += 1
            if not emitted_v:
                # v = attention @ last_h, emitted after block 0's matmuls so
                # the slower packed-weight DMA never blocks the Gram stream
                emitted_v = True
                v_ps = ms.tile([H, 2], f32, tag="misc")
                nc.tensor.matmul(v_ps[:, 0:1], attT[:, 0:H], lasth[:], start=True, stop=True)
                nc.tensor.matmul(v_ps[:, 1:2], attT[:, H:2 * H], lasth[:], start=True, stop=True)
                v_sb = wp.tile([H, 2], f32, tag="vsb")
                nc.vector.tensor_copy(v_sb[:], v_ps[:])

        # ---- NodeCell over p_cache ----
        h_fin = st["h"]
        hm_f = sb.tile([H, 1], f32, tag="hmf")
        nc.vector.tensor_reduce(hm_f[:], h_fin[:], mybir.AxisListType.X, Alu.add)
        hm_bf = sb.tile([H, 1], bf16, tag="hmb")
        nc.scalar.activation(hm_bf[:], hm_f[:], Act.Copy, scale=1.0 / P)

        rz_ps = ms.tile([H, 2], f32, tag="misc")
        nc.tensor.matmul(rz_ps[:, 0:1], swT[:, 0:H], xpk[:, 0:1], start=True, stop=False)
        nc.tensor.matmul(rz_ps[:, 0:1], swT[:, H:2 * H], xpk[:, 1:2], start=False, stop=False)
        nc.tensor.matmul(rz_ps[:, 0:1], swT[:, 2 * H:3 * H], hm_bf[:], start=False, stop=True)
        nc.tensor.matmul(rz_ps[:, 1:2], suT[:, 0:H], xpk[:, 0:1], start=True, stop=False)
        nc.tensor.matmul(rz_ps[:, 1:2], suT[:, H:2 * H], xpk[:, 1:2], start=False, stop=False)
        nc.tensor.matmul(rz_ps[:, 1:2], suT[:, 2 * H:3 * H], hm_bf[:], start=False, stop=True)
        nr_bf = sb.tile([H, 1], f32, tag="nr")
        nc.scalar.activation(nr_bf[:], rz_ps[:, 0:1], Act.Sigmoid, bias=swb[:])
        nz_bf = sb.tile([H, 1], f32, tag="nz")
        nc.scalar.activation(nz_bf[:], rz_ps[:, 1:2], Act.Sigmoid, bias=sub_t[:])

        cx_ps = ms.tile([H, 2], f32, tag="misc")
        nc.tensor.matmul(cx_ps[:, 0:1], stT[:, 0:H], xpk[:, 0:1], start=True, stop=False)
        nc.tensor.matmul(cx_ps[:, 0:1], stT[:, H:2 * H], xpk[:, 1:2], start=False, stop=True)
        tb2 = sb.tile([H, 1], f32, tag="tb2")
        nc.vector.tensor_tensor(tb2[:], cx_ps[:, 0:1], stb[:], Alu.add)
        rp2 = sb.tile([H, P], bf16, tag="rp2")
        nc.vector.tensor_scalar(rp2[:], h_fin[:], nr_bf[:], None, Alu.mult)
        tr_ps = ms.tile([H, P], f32, tag="misc")
        nc.tensor.matmul(tr_ps[:], stT[:, 2 * H:3 * H], rp2[:], start=True, stop=True)
        tr_bf = sb.tile([H, P], bf16, tag="trb")
        nc.scalar.activation(tr_bf[:], tr_ps[:], Act.Tanh, bias=tb2[:])
        tmp1 = sb.tile([H, P], bf16, tag="tmp1")
        nc.vector.tensor_tensor(tmp1[:], tr_bf[:], h_fin[:], Alu.subtract)
        tmp2 = sb.tile([H, P], bf16, tag="tmp2")
        nc.vector.tensor_scalar(tmp2[:], tmp1[:], nz_bf[:], None, Alu.mult)
        e_bf = sb.tile([H, P], bf16, tag="e")
        nc.vector.tensor_tensor(e_bf[:], h_fin[:], tmp2[:], Alu.add)
        reg_f = sb.tile([H, 1], f32, tag="regf")
        nc.vector.tensor_reduce(reg_f[:], e_bf[:], mybir.AxisListType.X, Alu.add)
        reg_bf = sb.tile([H, 1], bf16, tag="regb")
        nc.scalar.activation(reg_bf[:], reg_f[:], Act.Copy, scale=1.0 / P)

        # ---- Part A tail: wsum, total ----
        g1_sb = wp.tile([H, 2 * H + 1], f32, tag="g1sb")
        nc.vector.tensor_copy(g1_sb[:], G1[:])
        g2_sb = wp.tile([H, H + 1], f32, tag="g2sb")
        nc.vector.tensor_copy(g2_sb[:], G2[:])

        tp_ps = ms.tile([H, H], f32, tag="misc")
        nc.tensor.transpose(tp_ps[:], g1_sb[:, H:2 * H], ident[:])
        gduT = wp.tile([H, H], f32, tag="gduT")
        nc.vector.tensor_copy(gduT[:], tp_ps[:])

        wsum_ps = ms.tile([H, 2], f32, tag="misc")
        nc.tensor.matmul(wsum_ps[:, 0:1], g1_sb[:, 0:H], v_sb[:, 0:1], start=True, stop=False)
        nc.tensor.matmul(wsum_ps[:, 0:1], gduT[:], v_sb[:, 1:2], start=False, stop=True)
        nc.tensor.matmul(wsum_ps[:, 1:2], g1_sb[:, H:2 * H], v_sb[:, 0:1], start=True, stop=False)
        nc.tensor.matmul(wsum_ps[:, 1:2], g2_sb[:, 0:H], v_sb[:, 1:2], start=False, stop=True)
        wsum_bf = sb.tile([H, 2], bf16, tag="wsbf")
        nc.vector.tensor_copy(wsum_bf[:], wsum_ps[:])

        tot_ps = ms.tile([1, 2], f32, tag="misc")
        nc.tensor.matmul(tot_ps[:, 0:1], g1_sb[:, 2 * H:2 * H + 1], v_sb[:, 0:1], start=True, stop=False)
        nc.tensor.matmul(tot_ps[:, 0:1], g2_sb[:, H:H + 1], v_sb[:, 1:2], start=False, stop=True)
        tot_sb = sb.tile([1, 1], f32, tag="totsb")
        nc.vector.tensor_copy(tot_sb[:], tot_ps[:, 0:1])

        # ---- output head ----
        u1_ps = ms.tile([H, 2], f32, tag="misc")
        nc.tensor.matmul(u1_ps[:, 0:1], o1T[:, 0:H], wsum_bf[:, 0:1], start=True, stop=False)
        nc.tensor.matmul(u1_ps[:, 0:1], o1T[:, H:2 * H], wsum_bf[:, 1:2], start=False, stop=True)
        nc.tensor.matmul(u1_ps[:, 1:2], o1T[:, 2 * H:3 * H], reg_bf[:], start=True, stop=True)
        u1_bf = sb.tile([H, 1], bf16, tag="u1bf")
        nc.vector.tensor_copy(u1_bf[:], u1_ps[:, 0:1])
        u2_bf = sb.tile([H, 1], bf16, tag="u2bf")
        nc.scalar.activation(u2_bf[:], u1_ps[:, 1:2], Act.Identity, bias=o1b[:])

        out_sb = wp.tile([H, 4], f32, tag="outsb")
        nc.gpsimd.memset(out_sb[:, 2:4], 0.0)
        zy_ps = ms.tile([H, 2], f32, tag="misc")
        nc.tensor.matmul(zy_ps[:, 0:1], o2T[:], u1_bf[:], start=True, stop=True)
        nc.tensor.matmul(zy_ps[:, 1:2], o2T[:], u2_bf[:], start=True, stop=True)
        nc.vector.tensor_copy(out_sb[:, 0:1], zy_ps[:, 0:1])
        nc.scalar.activation(out_sb[:, 1:2], zy_ps[:, 1:2], Act.Identity, bias=o2b[:])
        nc.vector.tensor_copy(out_sb[0:1, 2:3], tot_sb[:])
        nc.sync.dma_start(out_d[:], out_sb[:])

    nc.compile()
    return nc


def _prep_inputs(inputs):
    """Build the per-core in_maps from the full-size numpy inputs."""
    d = inputs
    down = np.ascontiguousarray(np.asarray(d["down_states"], dtype=np.float32))
    up = np.ascontiguousarray(np.asarray(d["up_states"], dtype=np.float32))
    opi = int(np.asarray(d["op_idx"]))
    actions = np.asarray(d["actions"]).astype(np.int64)
    parent_idx = np.asarray(d["parent_idx"]).astype(np.int64)

    dpad = np.zeros((NTOT, H), np.float32)
    dpad[:N] = down
    upad = np.zeros((NTOT, H), np.float32)
    upad[:N] = up

    def t_bf(x):
        return np.ascontiguousarray(x.astype(BF16))

    def col(x):
        return np.ascontiguousarray(np.asarray(x, np.float32).reshape(-1, 1))

    def kblocks(wT, nb):
        # [nb*128, M] -> [128, nb*M] with k-block t at cols [t*M:(t+1)*M]
        M = wT.shape[1]
        return np.concatenate([wT[i * H:(i + 1) * H, :] for i in range(nb)], axis=1)

    attention = np.asarray(d["attention"], np.float32)
    emb = np.asarray(d["act_embed"], np.float32)[actions]        # [P, L, E]
    embT = emb.transpose(2, 1, 0).reshape(H, L * P)              # [E, L*P]
    h0T = down[parent_idx].T                                     # [H, P]
    x_pack = np.stack([down[opi], up[opi]], axis=1)              # [H, 2]

    # bf16 pack: order must match bseg() calls in _build_program
    wbp = np.concatenate([
        embT,
        h0T,
        kblocks(np.asarray(d["action_W"], np.float32).T, 2),
        kblocks(np.asarray(d["action_TW"], np.float32).T, 2),
        kblocks(np.asarray(d["sched_W"], np.float32).T, 3),
        kblocks(np.asarray(d["sched_U"], np.float32).T, 3),
        kblocks(np.asarray(d["sched_T"], np.float32).T, 3),
        kblocks(np.asarray(d["out1_W"], np.float32).T, 3),
        np.asarray(d["out2_W"], np.float32).T,
        x_pack,
    ], axis=1)
    # f32 pack: order must match fseg() calls
    wfp = np.concatenate([
        attention.T,
        col(d["last_h"]),
        np.asarray(d["action_b"], np.float32).reshape(2, H).T,
        col(d["action_Tb"]),
        col(d["sched_Wb"]),
        col(d["sched_Ub"]),
        col(d["sched_Tb"]),
        col(d["out1_b"]),
        col(d["out2_b"]),
    ], axis=1)
    common = {
        "wbp": t_bf(wbp),
        "wfp": np.ascontiguousarray(wfp.astype(np.float32)),
    }
    in_maps = []
    for c in range(NCORES):
        m = dict(common)
        m["dch"] = dpad[c * NPAD:(c + 1) * NPAD]
        m["uch"] = upad[c * NPAD:(c + 1) * NPAD]
        in_maps.append(m)
    return in_maps


_PROGRAM_CACHE = {}


def run(inputs, trace=False, **kw):
    from concourse import bass_utils

    if "nc" not in _PROGRAM_CACHE:
        _PROGRAM_CACHE["nc"] = _build_program()
    nc = _PROGRAM_CACHE["nc"]
    in_maps = _prep_inputs(inputs)
    res = bass_utils.run_bass_kernel_spmd(
        nc, in_maps, core_ids=list(range(NCORES)), trace=trace, **kw
    )
    parts = [r["partials"] for r in res.results]
    za = np.sum([p[:, 0] for p in parts], axis=0)
    yb = parts[0][:, 1]
    total = float(np.sum([p[0, 2] for p in parts]))
    logits = za / total + yb
    m = logits.max()
    e = np.exp(logits - m)
    out = (e / e.sum()).astype(np.float32)
    return out, res


def kernel(**inputs) -> np.ndarray:
    out, _ = run(inputs, trace=False)
    return out


# revision 18
# speedup vs baseline: 1.4352x; 1.4352x over previous
"""Trainium2 Bass kernel for the gnn_message_passing agent model.

Strategy (8 NeuronCores, SPMD):
  - Shard down_states/up_states row-wise across 8 cores (N=500000 padded to
    500736 = 8*62592 rows, zero rows are no-ops for all reductions).
  - Per core, stream the 62592x128 D and U chunks through SBUF in 4096-row
    blocks, cast to bf16, and accumulate Gram blocks on the TensorEngine:
        G1 = D^T @ [D | U | 1]   (PSUM, fp32 accumulate)  -> [G_dd | G_du | colsum_d]
        G2 = U^T @ [U | 1]                                -> [G_uu | colsum_u]
    Then  w = S v,  sum(w) = colsum . v,  w @ S = G v  follow from small
    tail matmuls with v = attention @ last_h.
  - The per-parent GRU (AxisCell) chains run replicated on every core in a
    transposed [feature, parent] layout, interleaved into the PE instruction
    stream so they overlap with the memory-bound streaming part.
  - The output head is linear up to the softmax:
        logits = (out2 @ out1_a @ wsum)/total + (out2 @ (out1_r@region + b1) + b2)
    Each core returns za = out2@out1_a@wsum_partial (additive across cores),
    yb (replicated), and total_partial. The host sums the partials
    (the cross-core all-reduce) and applies the final softmax on 128 values.
"""

import numpy as np
import ml_dtypes

BF16 = ml_dtypes.bfloat16

N, H, P, L, NA = 500000, 128, 64, 50, 128
NCORES = 8
NPAD = 62592            # per-core rows = 489 tiles of 128
NTOT = NPAD * NCORES    # 500736
SUB = 128               # rows per matmul sub-tile
BLOCK_B = 32            # sub-tiles per DMA block (4096 rows, 2 MiB fp32 read)
NT = NPAD // SUB        # 489 sub-tiles per core
GRU_EVERY = 9           # emit one GRU step per this many Gram sub-tiles


def _build_program():
    from contextlib import ExitStack

    import concourse.bass as bass
    import concourse.mybir as mybir
    import concourse.tile as tile
    from concourse import bacc
    from concourse.masks import make_identity

    dt = mybir.dt
    f32 = dt.float32
    bf16 = dt.bfloat16
    Act = mybir.ActivationFunctionType
    Alu = mybir.AluOpType

    nc = bacc.Bacc(
        "TRN2", target_bir_lowering=False, debug=False, num_devices=NCORES
    )

    # ---- DRAM I/O ----
    dch = nc.dram_tensor("dch", [NPAD, H], f32, kind="ExternalInput")
    uch = nc.dram_tensor("uch", [NPAD, H], f32, kind="ExternalInput")
    # all small tensors packed host-side into two [128, X] arrays
    WB = L * P + P + 4 * H + 2 * H + 4 * (3 * H) + H + 2   # bf16 pack cols
    WF = 2 * H + 9                                          # f32 pack cols
    wbp_d = nc.dram_tensor("wbp", [H, WB], bf16, kind="ExternalInput")
    wfp_d = nc.dram_tensor("wfp", [H, WF], f32, kind="ExternalInput")
    out_d = nc.dram_tensor("partials", [H, 4], f32, kind="ExternalOutput")

    with tile.TileContext(nc) as tc, ExitStack() as ctx:
        wp = ctx.enter_context(tc.tile_pool(name="wp", bufs=1))
        dp = ctx.enter_context(tc.tile_pool(name="dp", bufs=4))
        gb = ctx.enter_context(tc.tile_pool(name="gb", bufs=1, space="PSUM"))
        pb = ctx.enter_context(tc.tile_pool(name="pb", bufs=2, space="PSUM"))
        ms = ctx.enter_context(tc.tile_pool(name="ms", bufs=2, space="PSUM"))
        sb = ctx.enter_context(tc.tile_pool(name="sb", bufs=3))

        # ---- load small persistent tensors ----
        def wtile(dram, shape, dtype):
            t = wp.tile(shape, dtype, tag=dram.name)
            nc.sync.dma_start(t[:], dram[:].rearrange("a b -> a b") if False else dram[:])
            return t

        wbp = wp.tile([H, WB], bf16, tag="wbp")
        nc.sync.dma_start(wbp[:], wbp_d[:])
        wfp = wp.tile([H, WF], f32, tag="wfp")
        nc.sync.dma_start(wfp[:], wfp_d[:])

        def bseg(n):
            o = bseg.off
            bseg.off += n
            return wbp[:, o:o + n]
        bseg.off = 0
        embT = bseg(L * P)
        h0T = bseg(P)
        awT = bseg(4 * H)
        atwT = bseg(2 * H)
        swT = bseg(3 * H)
        suT = bseg(3 * H)
        stT = bseg(3 * H)
        o1T = bseg(3 * H)
        o2T = bseg(H)
        xpk = bseg(2)

        def fseg(n):
            o = fseg.off
            fseg.off += n
            return wfp[:, o:o + n]
        fseg.off = 0
        attT = fseg(2 * H)
        lasth = fseg(1)
        ab = fseg(2)
        atb = fseg(1)
        swb = fseg(1)
        sub_t = fseg(1)
        stb = fseg(1)
        o1b = fseg(1)
        o2b = fseg(1)

        ident = wp.tile([H, H], f32, tag="ident")
        make_identity(nc, ident[:])
        ones = wp.tile([H, 1], bf16, tag="ones")
        nc.vector.memset(ones[:], 1.0)

        # ---- persistent PSUM accumulators ----
        G1 = gb.tile([H, 2 * H + 1], f32, tag="G1")   # [G_dd | G_du | cs_d]
        G2 = gb.tile([H, H + 1], f32, tag="G2")       # [G_uu | cs_u]

        # ---- GRU chain, emitted as two half-steps interleaved with the
        #      Gram sub-tile stream so the PE never waits on ACT/DVE ----
        st = {"h": h0T, "l": 0, "a": None, "r": None, "z": None, "rp": None}

        def gru_half1():
            l, h_cur = st["l"], st["h"]
            x_l = embT[:, l * P:(l + 1) * P]
            a_ps = pb.tile([H, 2, P], f32, tag="a")
            nc.tensor.matmul(a_ps[:, 0, :], awT[:, 0:H], x_l, start=True, stop=False)
            nc.tensor.matmul(a_ps[:, 0, :], awT[:, 2 * H:3 * H], h_cur[:], start=False, stop=True)
            nc.tensor.matmul(a_ps[:, 1, :], awT[:, H:2 * H], x_l, start=True, stop=False)
            nc.tensor.matmul(a_ps[:, 1, :], awT[:, 3 * H:4 * H], h_cur[:], start=False, stop=True)
            r_bf = sb.tile([H, P], bf16, tag="r")
            nc.scalar.activation(r_bf[:], a_ps[:, 0, :], Act.Sigmoid, bias=ab[:, 0:1])
            rp = sb.tile([H, P], bf16, tag="rp")
            nc.vector.tensor_tensor(rp[:], r_bf[:], h_cur[:], Alu.mult)
            z_bf = sb.tile([H, P], bf16, tag="z")
            nc.scalar.activation(z_bf[:], a_ps[:, 1, :], Act.Sigmoid, bias=ab[:, 1:2])
            st["r"], st["z"], st["rp"] = r_bf, z_bf, rp

        def gru_half2():
            l, h_cur = st["l"], st["h"]
            x_l = embT[:, l * P:(l + 1) * P]
            t_ps = pb.tile([H, P], f32, tag="t")
            nc.tensor.matmul(t_ps[:], atwT[:, 0:H], x_l, start=True, stop=False)
            nc.tensor.matmul(t_ps[:], atwT[:, H:2 * H], st["rp"][:], start=False, stop=True)
            t_bf = sb.tile([H, P], bf16, tag="tb")
            nc.scalar.activation(t_bf[:], t_ps[:], Act.Tanh, bias=atb[:, 0:1])
            # h' = h + z*(t - h)
            tmh = sb.tile([H, P], bf16, tag="tmh")
            nc.vector.tensor_tensor(tmh[:], t_bf[:], h_cur[:], Alu.subtract)
            ztm = sb.tile([H, P], bf16, tag="ztm")
            nc.vector.tensor_tensor(ztm[:], st["z"][:], tmh[:], Alu.mult)
            h_new = sb.tile([H, P], bf16, tag="h")
            nc.vector.tensor_tensor(h_new[:], h_cur[:], ztm[:], Alu.add)
            st["h"] = h_new
            st["l"] += 1

        # schedule: step l half1 at sub-tile l*GRU_EVERY+10, half2 4 later
        # (offset 10 keeps the first blocks' Gram matmuls unblocked while the
        #  packed-weight DMA lands)
        gru_sched = {}
        for l in range(L):
            gru_sched[l * GRU_EVERY + 10] = gru_half1
            gru_sched[l * GRU_EVERY + 14] = gru_half2

        # ---- streaming Gram accumulation ----
        # small first block so the PE starts early, small last blocks so the
        # trailing PE work after the final DMA is short
        sizes = [8, 16] + [BLOCK_B] * ((NT - 48) // BLOCK_B) + [16, 8]
        sizes[2] += NT - sum(sizes)
        blocks = []
        s0 = 0
        for b in sizes:
            blocks.append((s0, b))
            s0 += b * SUB
        assert s0 == NPAD

        kidx = 0
        emitted_v = False
        for (s0, B) in blocks:
            # SWDGE DMA with inline fp32->bf16 cast into the packed S tile:
            # S[:, b, :] = [D_b | U_b | 1 1], so each sub-tile needs only two
            # wide matmuls (rhs N=257 and N=129).
            d_bf = dp.tile([SUB, B, H], bf16, tag="d")
            nc.gpsimd.dma_start(
                d_bf[:].rearrange("p b h -> p (b h)"),
                dch[s0:s0 + B * SUB, :].rearrange("(p b) h -> p (b h)", p=SUB),
            )
            u_bf = dp.tile([SUB, B, H], bf16, tag="u")
            nc.gpsimd.dma_start(
                u_bf[:].rearrange("p b h -> p (b h)"),
                uch[s0:s0 + B * SUB, :].rearrange("(p b) h -> p (b h)", p=SUB),
            )
            for b in range(B):
                st_, sp_ = kidx == 0, kidx == NT - 1
                nc.tensor.matmul(G1[:, 0:H], d_bf[:, b, :], d_bf[:, b, :], start=st_, stop=sp_)
                nc.tensor.matmul(G1[:, H:2 * H], d_bf[:, b, :], u_bf[:, b, :], start=st_, stop=sp_)
                nc.tensor.matmul(G1[:, 2 * H:2 * H + 1], d_bf[:, b, :], ones[:], start=st_, stop=sp_)
                nc.tensor.matmul(G2[:, 0:H], u_bf[:, b, :], u_bf[:, b, :], start=st_, stop=sp_)
                nc.tensor.matmul(G2[:, H:H + 1], u_bf[:, b, :], ones[:], start=st_, stop=sp_)
                act = gru_sched.get(kidx)
                if act is not None:
                    act()
                kidx += 1
            if not emitted_v:
                # v = attention @ last_h, emitted after block 0's matmuls so
                # the slower packed-weight DMA never blocks the Gram stream
                emitted_v = True
                v_ps = ms.tile([H, 2], f32, tag="misc")
                nc.tensor.matmul(v_ps[:, 0:1], attT[:, 0:H], lasth[:], start=True, stop=True)
                nc.tensor.matmul(v_ps[:, 1:2], attT[:, H:2 * H], lasth[:], start=True, stop=True)
                v_sb = wp.tile([H, 2], f32, tag="vsb")
                nc.vector.tensor_copy(v_sb[:], v_ps[:])

        # ---- NodeCell over p_cache ----
        h_fin = st["h"]
        hm_f = sb.tile([H, 1], f32, tag="hmf")
        nc.vector.tensor_reduce(hm_f[:], h_fin[:], mybir.AxisListType.X, Alu.add)
        hm_bf = sb.tile([H, 1], bf16, tag="hmb")
        nc.scalar.activation(hm_bf[:], hm_f[:], Act.Copy, scale=1.0 / P)

        rz_ps = ms.tile([H, 2], f32, tag="misc")
        nc.tensor.matmul(rz_ps[:, 0:1], swT[:, 0:H], xpk[:, 0:1], start=True, stop=False)
        nc.tensor.matmul(rz_ps[:, 0:1], swT[:, H:2 * H], xpk[:, 1:2], start=False, stop=False)
        nc.tensor.matmul(rz_ps[:, 0:1], swT[:, 2 * H:3 * H], hm_bf[:], start=False, stop=True)
        nc.tensor.matmul(rz_ps[:, 1:2], suT[:, 0:H], xpk[:, 0:1], start=True, stop=False)
        nc.tensor.matmul(rz_ps[:, 1:2], suT[:, H:2 * H], xpk[:, 1:2], start=False, stop=False)
        nc.tensor.matmul(rz_ps[:, 1:2], suT[:, 2 * H:3 * H], hm_bf[:], start=False, stop=True)
        nr_bf = sb.tile([H, 1], f32, tag="nr")
        nc.scalar.activation(nr_bf[:], rz_ps[:, 0:1], Act.Sigmoid, bias=swb[:])
        nz_bf = sb.tile([H, 1], f32, tag="nz")
        nc.scalar.activation(nz_bf[:], rz_ps[:, 1:2], Act.Sigmoid, bias=sub_t[:])

        cx_ps = ms.tile([H, 2], f32, tag="misc")
        nc.tensor.matmul(cx_ps[:, 0:1], stT[:, 0:H], xpk[:, 0:1], start=True, stop=False)
        nc.tensor.matmul(cx_ps[:, 0:1], stT[:, H:2 * H], xpk[:, 1:2], start=False, stop=True)
        tb2 = sb.tile([H, 1], f32, tag="tb2")
        nc.vector.tensor_tensor(tb2[:], cx_ps[:, 0:1], stb[:], Alu.add)
        rp2 = sb.tile([H, P], bf16, tag="rp2")
        nc.vector.tensor_scalar(rp2[:], h_fin[:], nr_bf[:], None, Alu.mult)
        tr_ps = ms.tile([H, P], f32, tag="misc")
        nc.tensor.matmul(tr_ps[:], stT[:, 2 * H:3 * H], rp2[:], start=True, stop=True)
        tr_bf = sb.tile([H, P], bf16, tag="trb")
        nc.scalar.activation(tr_bf[:], tr_ps[:], Act.Tanh, bias=tb2[:])
        tmp1 = sb.tile([H, P], bf16, tag="tmp1")
        nc.vector.tensor_tensor(tmp1[:], tr_bf[:], h_fin[:], Alu.subtract)
        tmp2 = sb.tile([H, P], bf16, tag="tmp2")
        nc.vector.tensor_scalar(tmp2[:], tmp1[:], nz_bf[:], None, Alu.mult)
        e_bf = sb.tile([H, P], bf16, tag="e")
        nc.vector.tensor_tensor(e_bf[:], h_fin[:], tmp2[:], Alu.add)
        reg_f = sb.tile([H, 1], f32, tag="regf")
        nc.vector.tensor_reduce(reg_f[:], e_bf[:], mybir.AxisListType.X, Alu.add)
        reg_bf = sb.tile([H, 1], bf16, tag="regb")
        nc.scalar.activation(reg_bf[:], reg_f[:], Act.Copy, scale=1.0 / P)

        # ---- Part A tail: wsum, total ----
        g1_sb = wp.tile([H, 2 * H + 1], f32, tag="g1sb")
        nc.vector.tensor_copy(g1_sb[:], G1[:])
        g2_sb = wp.tile([H, H + 1], f32, tag="g2sb")
        nc.vector.tensor_copy(g2_sb[:], G2[:])

        tp_ps = ms.tile([H, H], f32, tag="misc")
        nc.tensor.transpose(tp_ps[:], g1_sb[:, H:2 * H], ident[:])
        gduT = wp.tile([H, H], f32, tag="gduT")
        nc.vector.tensor_copy(gduT[:], tp_ps[:])

        wsum_ps = ms.tile([H, 2], f32, tag="misc")
        nc.tensor.matmul(wsum_ps[:, 0:1], g1_sb[:, 0:H], v_sb[:, 0:1], start=True, stop=False)
        nc.tensor.matmul(wsum_ps[:, 0:1], gduT[:], v_sb[:, 1:2], start=False, stop=True)
        nc.tensor.matmul(wsum_ps[:, 1:2], g1_sb[:, H:2 * H], v_sb[:, 0:1], start=True, stop=False)
        nc.tensor.matmul(wsum_ps[:, 1:2], g2_sb[:, 0:H], v_sb[:, 1:2], start=False, stop=True)
        wsum_bf = sb.tile([H, 2], bf16, tag="wsbf")
        nc.vector.tensor_copy(wsum_bf[:], wsum_ps[:])

        tot_ps = ms.tile([1, 2], f32, tag="misc")
        nc.tensor.matmul(tot_ps[:, 0:1], g1_sb[:, 2 * H:2 * H + 1], v_sb[:, 0:1], start=True, stop=False)
        nc.tensor.matmul(tot_ps[:, 0:1], g2_sb[:, H:H + 1], v_sb[:, 1:2], start=False, stop=True)
        tot_sb = sb.tile([1, 1], f32, tag="totsb")
        nc.vector.tensor_copy(tot_sb[:], tot_ps[:, 0:1])

        # ---- output head ----
        u1_ps = ms.tile([H, 2], f32, tag="misc")
        nc.tensor.matmul(u1_ps[:, 0:1], o1T[:, 0:H], wsum_bf[:, 0:1], start=True, stop=False)
        nc.tensor.matmul(u1_ps[:, 0:1], o1T[:, H:2 * H], wsum_bf[:, 1:2], start=False, stop=True)
        nc.tensor.matmul(u1_ps[:, 1:2], o1T[:, 2 * H:3 * H], reg_bf[:], start=True, stop=True)
        u1_bf = sb.tile([H, 1], bf16, tag="u1bf")
        nc.vector.tensor_copy(u1_bf[:], u1_ps[:, 0:1])
        u2_bf = sb.tile([H, 1], bf16, tag="u2bf")
        nc.scalar.activation(u2_bf[:], u1_ps[:, 1:2], Act.Identity, bias=o1b[:])

        out_sb = wp.tile([H, 4], f32, tag="outsb")
        nc.gpsimd.memset(out_sb[:, 2:4], 0.0)
        zy_ps = ms.tile([H, 2], f32, tag="misc")
        nc.tensor.matmul(zy_ps[:, 0:1], o2T[:], u1_bf[:], start=True, stop=True)
        nc.tensor.matmul(zy_ps[:, 1:2], o2T[:], u2_bf[:], start=True, stop=True)
        nc.vector.tensor_copy(out_sb[:, 0:1], zy_ps[:, 0:1])
        nc.scalar.activation(out_sb[:, 1:2], zy_ps[:, 1:2], Act.Identity, bias=o2b[:])
        nc.vector.tensor_copy(out_sb[0:1, 2:3], tot_sb[:])
        nc.sync.dma_start(out_d[:], out_sb[:])

    nc.compile()
    return nc


def _prep_inputs(inputs):
    """Build the per-core in_maps from the full-size numpy inputs."""
    d = inputs
    down = np.ascontiguousarray(np.asarray(d["down_states"], dtype=np.float32))
    up = np.ascontiguousarray(np.asarray(d["up_states"], dtype=np.float32))
    opi = int(np.asarray(d["op_idx"]))
    actions = np.asarray(d["actions"]).astype(np.int64)
    parent_idx = np.asarray(d["parent_idx"]).astype(np.int64)

    dpad = np.zeros((NTOT, H), np.float32)
    dpad[:N] = down
    upad = np.zeros((NTOT, H), np.float32)
    upad[:N] = up

    def t_bf(x):
        return np.ascontiguousarray(x.astype(BF16))

    def col(x):
        return np.ascontiguousarray(np.asarray(x, np.float32).reshape(-1, 1))

    def kblocks(wT, nb):
        # [nb*128, M] -> [128, nb*M] with k-block t at cols [t*M:(t+1)*M]
        M = wT.shape[1]
        return np.concatenate([wT[i * H:(i + 1) * H, :] for i in range(nb)], axis=1)

    attention = np.asarray(d["attention"], np.float32)
    emb = np.asarray(d["act_embed"], np.float32)[actions]        # [P, L, E]
    embT = emb.transpose(2, 1, 0).reshape(H, L * P)              # [E, L*P]
    h0T = down[parent_idx].T                                     # [H, P]
    x_pack = np.stack([down[opi], up[opi]], axis=1)              # [H, 2]

    # bf16 pack: order must match bseg() calls in _build_program
    wbp = np.concatenate([
        embT,
        h0T,
        kblocks(np.asarray(d["action_W"], np.float32).T, 2),
        kblocks(np.asarray(d["action_TW"], np.float32).T, 2),
        kblocks(np.asarray(d["sched_W"], np.float32).T, 3),
        kblocks(np.asarray(d["sched_U"], np.float32).T, 3),
        kblocks(np.asarray(d["sched_T"], np.float32).T, 3),
        kblocks(np.asarray(d["out1_W"], np.float32).T, 3),
        np.asarray(d["out2_W"], np.float32).T,
        x_pack,
    ], axis=1)
    # f32 pack: order must match fseg() calls
    wfp = np.concatenate([
        attention.T,
        col(d["last_h"]),
        np.asarray(d["action_b"], np.float32).reshape(2, H).T,
        col(d["action_Tb"]),
        col(d["sched_Wb"]),
        col(d["sched_Ub"]),
        col(d["sched_Tb"]),
        col(d["out1_b"]),
        col(d["out2_b"]),
    ], axis=1)
    common = {
        "wbp": t_bf(wbp),
        "wfp": np.ascontiguousarray(wfp.astype(np.float32)),
    }
    in_maps = []
    for c in range(NCORES):
        m = dict(common)
        m["dch"] = dpad[c * NPAD:(c + 1) * NPAD]
        m["uch"] = upad[c * NPAD:(c + 1) * NPAD]
        in_maps.append(m)
    return in_maps


_PROGRAM_CACHE = {}


def run(inputs, trace=False, **kw):
    from concourse import bass_utils

    if "nc" not in _PROGRAM_CACHE:
        _PROGRAM_CACHE["nc"] = _build_program()
    nc = _PROGRAM_CACHE["nc"]
    in_maps = _prep_inputs(inputs)
    res = bass_utils.run_bass_kernel_spmd(
        nc, in_maps, core_ids=list(range(NCORES)), trace=trace, **kw
    )
    parts = [r["partials"] for r in res.results]
    za = np.sum([p[:, 0] for p in parts], axis=0)
    yb = parts[0][:, 1]
    total = float(np.sum([p[0, 2] for p in parts]))
    logits = za / total + yb
    m = logits.max()
    e = np.exp(logits - m)
    out = (e / e.sum()).astype(np.float32)
    return out, res


def kernel(**inputs) -> np.ndarray:
    out, _ = run(inputs, trace=False)
    return out


# revision 19
# speedup vs baseline: 1.5161x; 1.0564x over previous
"""Trainium2 Bass kernel for the gnn_message_passing agent model.

Strategy (8 NeuronCores, SPMD):
  - Shard down_states/up_states row-wise across 8 cores (N=500000 padded to
    500736 = 8*62592 rows, zero rows are no-ops for all reductions).
  - Per core, stream the 62592x128 D and U chunks through SBUF in 4096-row
    blocks, cast to bf16, and accumulate Gram blocks on the TensorEngine:
        G1 = D^T @ [D | U | 1]   (PSUM, fp32 accumulate)  -> [G_dd | G_du | colsum_d]
        G2 = U^T @ [U | 1]                                -> [G_uu | colsum_u]
    Then  w = S v,  sum(w) = colsum . v,  w @ S = G v  follow from small
    tail matmuls with v = attention @ last_h.
  - The per-parent GRU (AxisCell) chains run replicated on every core in a
    transposed [feature, parent] layout, interleaved into the PE instruction
    stream so they overlap with the memory-bound streaming part.
  - The output head is linear up to the softmax:
        logits = (out2 @ out1_a @ wsum)/total + (out2 @ (out1_r@region + b1) + b2)
    Each core returns za = out2@out1_a@wsum_partial (additive across cores),
    yb (replicated), and total_partial. The host sums the partials
    (the cross-core all-reduce) and applies the final softmax on 128 values.
"""

import numpy as np
import ml_dtypes

BF16 = ml_dtypes.bfloat16

N, H, P, L, NA = 500000, 128, 64, 50, 128
NCORES = 8
NPAD = 62592            # per-core rows = 489 tiles of 128
NTOT = NPAD * NCORES    # 500736
SUB = 128               # rows per matmul sub-tile
BLOCK_B = 32            # sub-tiles per DMA block (4096 rows, 2 MiB fp32 read)
NT = NPAD // SUB        # 489 sub-tiles per core
GRU_EVERY = 9           # emit one GRU step per this many Gram sub-tiles


def _build_program():
    from contextlib import ExitStack

    import concourse.bass as bass
    import concourse.mybir as mybir
    import concourse.tile as tile
    from concourse import bacc
    from concourse.masks import make_identity

    dt = mybir.dt
    f32 = dt.float32
    bf16 = dt.bfloat16
    Act = mybir.ActivationFunctionType
    Alu = mybir.AluOpType

    nc = bacc.Bacc(
        "TRN2", target_bir_lowering=False, debug=False, num_devices=NCORES
    )

    # ---- DRAM I/O ----
    dch = nc.dram_tensor("dch", [NPAD, H], f32, kind="ExternalInput")
    uch = nc.dram_tensor("uch", [NPAD, H], f32, kind="ExternalInput")
    # all small tensors packed host-side into two [128, X] arrays
    WB = L * P + P + 4 * H + 2 * H + 4 * (3 * H) + H + 2   # bf16 pack cols
    WF = 2 * H + 9                                          # f32 pack cols
    wbp_d = nc.dram_tensor("wbp", [H, WB], bf16, kind="ExternalInput")
    wfp_d = nc.dram_tensor("wfp", [H, WF], f32, kind="ExternalInput")
    out_d = nc.dram_tensor("partials", [H, 4], f32, kind="ExternalOutput")

    with tile.TileContext(nc) as tc, ExitStack() as ctx:
        wp = ctx.enter_context(tc.tile_pool(name="wp", bufs=1))
        dp = ctx.enter_context(tc.tile_pool(name="dp", bufs=4))
        gb = ctx.enter_context(tc.tile_pool(name="gb", bufs=1, space="PSUM"))
        pb = ctx.enter_context(tc.tile_pool(name="pb", bufs=2, space="PSUM"))
        ms = ctx.enter_context(tc.tile_pool(name="ms", bufs=2, space="PSUM"))
        sb = ctx.enter_context(tc.tile_pool(name="sb", bufs=3))

        # ---- load small persistent tensors ----
        def wtile(dram, shape, dtype):
            t = wp.tile(shape, dtype, tag=dram.name)
            nc.sync.dma_start(t[:], dram[:].rearrange("a b -> a b") if False else dram[:])
            return t

        wbp = wp.tile([H, WB], bf16, tag="wbp")
        nc.sync.dma_start(wbp[:], wbp_d[:])
        wfp = wp.tile([H, WF], f32, tag="wfp")
        nc.sync.dma_start(wfp[:], wfp_d[:])

        def bseg(n):
            o = bseg.off
            bseg.off += n
            return wbp[:, o:o + n]
        bseg.off = 0
        embT = bseg(L * P)
        h0T = bseg(P)
        awT = bseg(4 * H)
        atwT = bseg(2 * H)
        swT = bseg(3 * H)
        suT = bseg(3 * H)
        stT = bseg(3 * H)
        o1T = bseg(3 * H)
        o2T = bseg(H)
        xpk = bseg(2)

        def fseg(n):
            o = fseg.off
            fseg.off += n
            return wfp[:, o:o + n]
        fseg.off = 0
        attT = fseg(2 * H)
        lasth = fseg(1)
        ab = fseg(2)
        atb = fseg(1)
        swb = fseg(1)
        sub_t = fseg(1)
        stb = fseg(1)
        o1b = fseg(1)
        o2b = fseg(1)

        ones = wp.tile([H, 1], bf16, tag="ones")
        nc.vector.memset(ones[:], 1.0)

        # ---- persistent PSUM accumulators ----
        G1 = gb.tile([H, 2 * H + 1], f32, tag="G1")   # [G_dd | G_du | cs_d]
        G2 = gb.tile([H, H + 1], f32, tag="G2")       # [G_uu | cs_u]

        # ---- GRU chain, emitted as two half-steps interleaved with the
        #      Gram sub-tile stream so the PE never waits on ACT/DVE ----
        st = {"h": h0T, "l": 0, "a": None, "r": None, "z": None, "rp": None}

        def gru_half1():
            l, h_cur = st["l"], st["h"]
            x_l = embT[:, l * P:(l + 1) * P]
            a_ps = pb.tile([H, 2, P], f32, tag="a")
            nc.tensor.matmul(a_ps[:, 0, :], awT[:, 0:H], x_l, start=True, stop=False)
            nc.tensor.matmul(a_ps[:, 0, :], awT[:, 2 * H:3 * H], h_cur[:], start=False, stop=True)
            nc.tensor.matmul(a_ps[:, 1, :], awT[:, H:2 * H], x_l, start=True, stop=False)
            nc.tensor.matmul(a_ps[:, 1, :], awT[:, 3 * H:4 * H], h_cur[:], start=False, stop=True)
            r_bf = sb.tile([H, P], bf16, tag="r")
            nc.scalar.activation(r_bf[:], a_ps[:, 0, :], Act.Sigmoid, bias=ab[:, 0:1])
            rp = sb.tile([H, P], bf16, tag="rp")
            nc.vector.tensor_tensor(rp[:], r_bf[:], h_cur[:], Alu.mult)
            z_bf = sb.tile([H, P], bf16, tag="z")
            nc.scalar.activation(z_bf[:], a_ps[:, 1, :], Act.Sigmoid, bias=ab[:, 1:2])
            st["r"], st["z"], st["rp"] = r_bf, z_bf, rp

        def gru_half2():
            l, h_cur = st["l"], st["h"]
            x_l = embT[:, l * P:(l + 1) * P]
            t_ps = pb.tile([H, P], f32, tag="t")
            nc.tensor.matmul(t_ps[:], atwT[:, 0:H], x_l, start=True, stop=False)
            nc.tensor.matmul(t_ps[:], atwT[:, H:2 * H], st["rp"][:], start=False, stop=True)
            t_bf = sb.tile([H, P], bf16, tag="tb")
            nc.scalar.activation(t_bf[:], t_ps[:], Act.Tanh, bias=atb[:, 0:1])
            # h' = h + z*(t - h)
            tmh = sb.tile([H, P], bf16, tag="tmh")
            nc.vector.tensor_tensor(tmh[:], t_bf[:], h_cur[:], Alu.subtract)
            ztm = sb.tile([H, P], bf16, tag="ztm")
            nc.vector.tensor_tensor(ztm[:], st["z"][:], tmh[:], Alu.mult)
            h_new = sb.tile([H, P], bf16, tag="h")
            nc.vector.tensor_tensor(h_new[:], h_cur[:], ztm[:], Alu.add)
            st["h"] = h_new
            st["l"] += 1

        # schedule: step l half1 at sub-tile l*GRU_EVERY+10, half2 4 later
        # (offset 10 keeps the first blocks' Gram matmuls unblocked while the
        #  packed-weight DMA lands)
        gru_sched = {}
        for l in range(L):
            gru_sched[l * GRU_EVERY + 10] = gru_half1
            gru_sched[l * GRU_EVERY + 14] = gru_half2

        # ---- streaming Gram accumulation ----
        # small first block so the PE starts early, small last blocks so the
        # trailing PE work after the final DMA is short
        sizes = [8, 16] + [BLOCK_B] * ((NT - 48) // BLOCK_B) + [16, 8]
        sizes[2] += NT - sum(sizes)
        blocks = []
        s0 = 0
        for b in sizes:
            blocks.append((s0, b))
            s0 += b * SUB
        assert s0 == NPAD

        kidx = 0
        emitted_v = False
        for (s0, B) in blocks:
            # SWDGE DMA with inline fp32->bf16 cast into the packed S tile:
            # S[:, b, :] = [D_b | U_b | 1 1], so each sub-tile needs only two
            # wide matmuls (rhs N=257 and N=129).
            d_bf = dp.tile([SUB, B, H], bf16, tag="d")
            nc.gpsimd.dma_start(
                d_bf[:].rearrange("p b h -> p (b h)"),
                dch[s0:s0 + B * SUB, :].rearrange("(p b) h -> p (b h)", p=SUB),
            )
            u_bf = dp.tile([SUB, B, H], bf16, tag="u")
            nc.gpsimd.dma_start(
                u_bf[:].rearrange("p b h -> p (b h)"),
                uch[s0:s0 + B * SUB, :].rearrange("(p b) h -> p (b h)", p=SUB),
            )
            for b in range(B):
                st_, sp_ = kidx == 0, kidx == NT - 1
                nc.tensor.matmul(G1[:, 0:H], d_bf[:, b, :], d_bf[:, b, :], start=st_, stop=sp_)
                nc.tensor.matmul(G1[:, H:2 * H], d_bf[:, b, :], u_bf[:, b, :], start=st_, stop=sp_)
                nc.tensor.matmul(G1[:, 2 * H:2 * H + 1], d_bf[:, b, :], ones[:], start=st_, stop=sp_)
                nc.tensor.matmul(G2[:, 0:H], u_bf[:, b, :], u_bf[:, b, :], start=st_, stop=sp_)
                nc.tensor.matmul(G2[:, H:H + 1], u_bf[:, b, :], ones[:], start=st_, stop=sp_)
                act = gru_sched.get(kidx)
                if act is not None:
                    act()
                kidx += 1
            if not emitted_v:
                # v = attention @ last_h, emitted after block 0's matmuls so
                # the slower packed-weight DMA never blocks the Gram stream
                emitted_v = True
                v_ps = ms.tile([H, 2], f32, tag="misc")
                nc.tensor.matmul(v_ps[:, 0:1], attT[:, 0:H], lasth[:], start=True, stop=True)
                nc.tensor.matmul(v_ps[:, 1:2], attT[:, H:2 * H], lasth[:], start=True, stop=True)
                v_sb = wp.tile([H, 2], f32, tag="vsb")
                nc.vector.tensor_copy(v_sb[:], v_ps[:])

        ident = wp.tile([H, H], f32, tag="ident")
        make_identity(nc, ident[:])

        # ---- NodeCell over p_cache ----
        h_fin = st["h"]
        hm_f = sb.tile([H, 1], f32, tag="hmf")
        nc.vector.tensor_reduce(hm_f[:], h_fin[:], mybir.AxisListType.X, Alu.add)
        hm_bf = sb.tile([H, 1], bf16, tag="hmb")
        nc.scalar.activation(hm_bf[:], hm_f[:], Act.Copy, scale=1.0 / P)

        rz_ps = ms.tile([H, 2], f32, tag="misc")
        nc.tensor.matmul(rz_ps[:, 0:1], swT[:, 0:H], xpk[:, 0:1], start=True, stop=False)
        nc.tensor.matmul(rz_ps[:, 0:1], swT[:, H:2 * H], xpk[:, 1:2], start=False, stop=False)
        nc.tensor.matmul(rz_ps[:, 0:1], swT[:, 2 * H:3 * H], hm_bf[:], start=False, stop=True)
        nc.tensor.matmul(rz_ps[:, 1:2], suT[:, 0:H], xpk[:, 0:1], start=True, stop=False)
        nc.tensor.matmul(rz_ps[:, 1:2], suT[:, H:2 * H], xpk[:, 1:2], start=False, stop=False)
        nc.tensor.matmul(rz_ps[:, 1:2], suT[:, 2 * H:3 * H], hm_bf[:], start=False, stop=True)
        nr_bf = sb.tile([H, 1], f32, tag="nr")
        nc.scalar.activation(nr_bf[:], rz_ps[:, 0:1], Act.Sigmoid, bias=swb[:])
        nz_bf = sb.tile([H, 1], f32, tag="nz")
        nc.scalar.activation(nz_bf[:], rz_ps[:, 1:2], Act.Sigmoid, bias=sub_t[:])

        cx_ps = ms.tile([H, 2], f32, tag="misc")
        nc.tensor.matmul(cx_ps[:, 0:1], stT[:, 0:H], xpk[:, 0:1], start=True, stop=False)
        nc.tensor.matmul(cx_ps[:, 0:1], stT[:, H:2 * H], xpk[:, 1:2], start=False, stop=True)
        tb2 = sb.tile([H, 1], f32, tag="tb2")
        nc.vector.tensor_tensor(tb2[:], cx_ps[:, 0:1], stb[:], Alu.add)
        rp2 = sb.tile([H, P], bf16, tag="rp2")
        nc.vector.tensor_scalar(rp2[:], h_fin[:], nr_bf[:], None, Alu.mult)
        tr_ps = ms.tile([H, P], f32, tag="misc")
        nc.tensor.matmul(tr_ps[:], stT[:, 2 * H:3 * H], rp2[:], start=True, stop=True)
        tr_bf = sb.tile([H, P], bf16, tag="trb")
        nc.scalar.activation(tr_bf[:], tr_ps[:], Act.Tanh, bias=tb2[:])
        tmp1 = sb.tile([H, P], bf16, tag="tmp1")
        nc.vector.tensor_tensor(tmp1[:], tr_bf[:], h_fin[:], Alu.subtract)
        tmp2 = sb.tile([H, P], bf16, tag="tmp2")
        nc.vector.tensor_scalar(tmp2[:], tmp1[:], nz_bf[:], None, Alu.mult)
        e_bf = sb.tile([H, P], bf16, tag="e")
        nc.vector.tensor_tensor(e_bf[:], h_fin[:], tmp2[:], Alu.add)
        reg_f = sb.tile([H, 1], f32, tag="regf")
        nc.vector.tensor_reduce(reg_f[:], e_bf[:], mybir.AxisListType.X, Alu.add)
        reg_bf = sb.tile([H, 1], bf16, tag="regb")
        nc.scalar.activation(reg_bf[:], reg_f[:], Act.Copy, scale=1.0 / P)

        # ---- Part A tail: wsum, total ----
        # split the PSUM->SBUF copies across engines so they run in parallel;
        # the G_du copy goes first since the transpose depends only on it
        g1_sb = wp.tile([H, 2 * H + 1], f32, tag="g1sb")
        nc.vector.tensor_copy(g1_sb[:, H:2 * H], G1[:, H:2 * H])
        nc.scalar.copy(g1_sb[:, 0:H], G1[:, 0:H])
        nc.scalar.copy(g1_sb[:, 2 * H:2 * H + 1], G1[:, 2 * H:2 * H + 1])
        g2_sb = wp.tile([H, H + 1], f32, tag="g2sb")
        nc.vector.tensor_copy(g2_sb[:], G2[:])

        tp_ps = ms.tile([H, H], f32, tag="misc")
        nc.tensor.transpose(tp_ps[:], g1_sb[:, H:2 * H], ident[:])
        gduT = wp.tile([H, H], f32, tag="gduT")
        nc.vector.tensor_copy(gduT[:], tp_ps[:])

        wsum_ps = ms.tile([H, 2], f32, tag="misc")
        nc.tensor.matmul(wsum_ps[:, 0:1], g1_sb[:, 0:H], v_sb[:, 0:1], start=True, stop=False)
        nc.tensor.matmul(wsum_ps[:, 0:1], gduT[:], v_sb[:, 1:2], start=False, stop=True)
        nc.tensor.matmul(wsum_ps[:, 1:2], g1_sb[:, H:2 * H], v_sb[:, 0:1], start=True, stop=False)
        nc.tensor.matmul(wsum_ps[:, 1:2], g2_sb[:, 0:H], v_sb[:, 1:2], start=False, stop=True)
        wsum_bf = sb.tile([H, 2], bf16, tag="wsbf")
        nc.vector.tensor_copy(wsum_bf[:], wsum_ps[:])

        tot_ps = ms.tile([1, 2], f32, tag="misc")
        nc.tensor.matmul(tot_ps[:, 0:1], g1_sb[:, 2 * H:2 * H + 1], v_sb[:, 0:1], start=True, stop=False)
        nc.tensor.matmul(tot_ps[:, 0:1], g2_sb[:, H:H + 1], v_sb[:, 1:2], start=False, stop=True)
        tot_sb = sb.tile([1, 1], f32, tag="totsb")
        nc.vector.tensor_copy(tot_sb[:], tot_ps[:, 0:1])

        # ---- output head ----
        u1_ps = ms.tile([H, 2], f32, tag="misc")
        nc.tensor.matmul(u1_ps[:, 0:1], o1T[:, 0:H], wsum_bf[:, 0:1], start=True, stop=False)
        nc.tensor.matmul(u1_ps[:, 0:1], o1T[:, H:2 * H], wsum_bf[:, 1:2], start=False, stop=True)
        nc.tensor.matmul(u1_ps[:, 1:2], o1T[:, 2 * H:3 * H], reg_bf[:], start=True, stop=True)
        u1_bf = sb.tile([H, 1], bf16, tag="u1bf")
        nc.vector.tensor_copy(u1_bf[:], u1_ps[:, 0:1])
        u2_bf = sb.tile([H, 1], bf16, tag="u2bf")
        nc.scalar.activation(u2_bf[:], u1_ps[:, 1:2], Act.Identity, bias=o1b[:])

        out_sb = wp.tile([H, 4], f32, tag="outsb")
        nc.gpsimd.memset(out_sb[:, 2:4], 0.0)
        zy_ps = ms.tile([H, 2], f32, tag="misc")
        nc.tensor.matmul(zy_ps[:, 0:1], o2T[:], u1_bf[:], start=True, stop=True)
        nc.tensor.matmul(zy_ps[:, 1:2], o2T[:], u2_bf[:], start=True, stop=True)
        nc.vector.tensor_copy(out_sb[:, 0:1], zy_ps[:, 0:1])
        nc.scalar.activation(out_sb[:, 1:2], zy_ps[:, 1:2], Act.Identity, bias=o2b[:])
        nc.vector.tensor_copy(out_sb[0:1, 2:3], tot_sb[:])
        nc.sync.dma_start(out_d[:], out_sb[:])

    nc.compile()
    return nc


def _prep_inputs(inputs):
    """Build the per-core in_maps from the full-size numpy inputs."""
    d = inputs
    down = np.ascontiguousarray(np.asarray(d["down_states"], dtype=np.float32))
    up = np.ascontiguousarray(np.asarray(d["up_states"], dtype=np.float32))
    opi = int(np.asarray(d["op_idx"]))
    actions = np.asarray(d["actions"]).astype(np.int64)
    parent_idx = np.asarray(d["parent_idx"]).astype(np.int64)

    dpad = np.zeros((NTOT, H), np.float32)
    dpad[:N] = down
    upad = np.zeros((NTOT, H), np.float32)
    upad[:N] = up

    def t_bf(x):
        return np.ascontiguousarray(x.astype(BF16))

    def col(x):
        return np.ascontiguousarray(np.asarray(x, np.float32).reshape(-1, 1))

    def kblocks(wT, nb):
        # [nb*128, M] -> [128, nb*M] with k-block t at cols [t*M:(t+1)*M]
        M = wT.shape[1]
        return np.concatenate([wT[i * H:(i + 1) * H, :] for i in range(nb)], axis=1)

    attention = np.asarray(d["attention"], np.float32)
    emb = np.asarray(d["act_embed"], np.float32)[actions]        # [P, L, E]
    embT = emb.transpose(2, 1, 0).reshape(H, L * P)              # [E, L*P]
    h0T = down[parent_idx].T                                     # [H, P]
    x_pack = np.stack([down[opi], up[opi]], axis=1)              # [H, 2]

    # bf16 pack: order must match bseg() calls in _build_program
    wbp = np.concatenate([
        embT,
        h0T,
        kblocks(np.asarray(d["action_W"], np.float32).T, 2),
        kblocks(np.asarray(d["action_TW"], np.float32).T, 2),
        kblocks(np.asarray(d["sched_W"], np.float32).T, 3),
        kblocks(np.asarray(d["sched_U"], np.float32).T, 3),
        kblocks(np.asarray(d["sched_T"], np.float32).T, 3),
        kblocks(np.asarray(d["out1_W"], np.float32).T, 3),
        np.asarray(d["out2_W"], np.float32).T,
        x_pack,
    ], axis=1)
    # f32 pack: order must match fseg() calls
    wfp = np.concatenate([
        attention.T,
        col(d["last_h"]),
        np.asarray(d["action_b"], np.float32).reshape(2, H).T,
        col(d["action_Tb"]),
        col(d["sched_Wb"]),
        col(d["sched_Ub"]),
        col(d["sched_Tb"]),
        col(d["out1_b"]),
        col(d["out2_b"]),
    ], axis=1)
    common = {
        "wbp": t_bf(wbp),
        "wfp": np.ascontiguousarray(wfp.astype(np.float32)),
    }
    in_maps = []
    for c in range(NCORES):
        m = dict(common)
        m["dch"] = dpad[c * NPAD:(c + 1) * NPAD]
        m["uch"] = upad[c * NPAD:(c + 1) * NPAD]
        in_maps.append(m)
    return in_maps


_PROGRAM_CACHE = {}


def run(inputs, trace=False, **kw):
    from concourse import bass_utils

    if "nc" not in _PROGRAM_CACHE:
        _PROGRAM_CACHE["nc"] = _build_program()
    nc = _PROGRAM_CACHE["nc"]
    in_maps = _prep_inputs(inputs)
    res = bass_utils.run_bass_kernel_spmd(
        nc, in_maps, core_ids=list(range(NCORES)), trace=trace, **kw
    )
    parts = [r["partials"] for r in res.results]
    za = np.sum([p[:, 0] for p in parts], axis=0)
    yb = parts[0][:, 1]
    total = float(np.sum([p[0, 2] for p in parts]))
    logits = za / total + yb
    m = logits.max()
    e = np.exp(logits - m)
    out = (e / e.sum()).astype(np.float32)
    return out, res


def kernel(**inputs) -> np.ndarray:
    out, _ = run(inputs, trace=False)
    return out
